# revision 19
# baseline (speedup 1.0000x reference)
"""ConvGRU Trainium2 kernel (nn_ConvRnn): B=4, T=8, C_in=C_out=64, H=W=96, 3x3 SAME.

Strategy (v2 — fp16 end-to-end to minimize PJRT operand bytes):
- 8 cores = 4 samples x 2 height-halves. Bottom halves are row-flipped on the
  host (weights row-flipped too) so a single SPMD program serves all cores.
- No cross-core communication: each core computes a shrinking extended region
  R_t = 48 + 2*(7-t) rows so the halo needed by later steps is computed
  redundantly (avg +15% compute, zero sync).
- Everything on SBUF is fp16 (x, h, weights, gates); PSUM accumulates fp32 and
  activations apply fp32 biases. Measured axon dispatch cost is ~0.6 ms per MB
  of bound input operands per core, so halving bytes is the dominant win; the
  on-device exec is only a few hundred us.
- x arrives unpadded and row-packed [C,456,96] fp16: step t only reads its
  shrinking window of R_t+2 rows, so per-step slices are concatenated on the
  host (sum_t R_t+2 = 456 < 8*64). Zero padding rows/cols live in SBUF tiles
  zero-filled once by memset (no 'zeros' input tensor).
- Convs are 9 shifted fp16 matmuls (K=128 channels = [x|h], M=out channels,
  N<=512 pixels) accumulating in PSUM. Width padded to 98 with zero columns.
- Layout: channel c of x at partition c (0:64); h/r/rh/z/h_tilde at partition
  64+c so every 2-input DVE op has equal base partitions.
- Per step: rz-conv -> sigmoid (r into XRH, z into Zt) -> rh=r*h in-place in
  XRH -> h-conv -> tanh -> d=h~-h, p=z*d (in-place in HT) -> h_new = h+p into
  next XH; h_new rows [2,50) DMA'd to fp16 output.
"""
import numpy as np

import concourse.bacc as bacc
import concourse.tile as tile
from concourse import mybir

F32 = mybir.dt.float32
F16 = mybir.dt.float16
AF = mybir.ActivationFunctionType
ALU = mybir.AluOpType

B, T, C, HW = 4, 8, 64, 96
W = 96
WP = 98          # padded width
WIN = 66         # rows per shard window (incl. 2 top pad rows)
XOFF = [0]       # per-step row offsets into the packed xs input
for _t in range(8):
    XOFF.append(XOFF[-1] + 50 + 2 * (7 - _t))
XROWS = XOFF[-1]  # 456 packed rows total
NCORES = 8
PSUM_ROWS = 10   # rows per PSUM chunk (2 banks)
DVE_ROWS = 16    # rows per DVE elementwise chunk
DMA_ROWS = 17    # rows per x-load DMA piece


def _r_of(t):
    return 48 + 2 * (7 - t)


def build_program(reps=1, hw_loop=False, dynamic_reps=False):
    """reps>1 repeats the full body (incl. state zero-fill) on-device so a
    timing harness can measure steady-state per-execution HW time as the
    slope of wall time vs reps. hw_loop uses a hardware For_i (constant
    program size; an all-engine barrier separates iterations); dynamic_reps
    makes the loop bound a runtime [1,1] uint32 input named "reps" so one
    NEFF serves every rep count. kernel() always uses reps=1."""
    nc = bacc.Bacc("TRN2", target_bir_lowering=False, debug=False,
                   enable_asserts=False, num_devices=NCORES)
    reps_d = None
    if dynamic_reps:
        hw_loop = True
        reps_d = nc.dram_tensor("reps", [1, 1], mybir.dt.uint32,
                                kind="ExternalInput")
    xs_d = nc.dram_tensor("xs", [C, XROWS, W], F16, kind="ExternalInput").ap()
    w_d = nc.dram_tensor("w", [9, 128, 192], F16, kind="ExternalInput").ap()
    b_d = nc.dram_tensor("b", [128, 2], F32, kind="ExternalInput").ap()
    out_d = nc.dram_tensor("out", [T, C, 48, W], F16, kind="ExternalOutput").ap()
    # Chaining token: lets a timing harness thread a data dependency through
    # repeated executions so XLA cannot CSE/DCE the repeats.
    tok_d = nc.dram_tensor("tok", [1, 1], F32, kind="ExternalInput").ap()
    toko_d = nc.dram_tensor("toko", [1, 1], F32, kind="ExternalOutput").ap()

    with tile.TileContext(nc) as tc:
        with tc.tile_pool(name="persist", bufs=1) as pp, \
             tc.tile_pool(name="prz", bufs=2, space="PSUM") as prz, \
             tc.tile_pool(name="ph", bufs=2, space="PSUM") as ph:
            w_t = pp.tile([128, 9, 192], F16, name="w")
            b_t = pp.tile([128, 2], F32, name="b")
            tok_t = pp.tile([1, 1], F32, name="tok")

            # Persistent double-buffered tiles (explicit, so zero-fill persists)
            xh = [pp.tile([128, WIN, WP], F16, name=f"xh{i}") for i in range(2)]
            xrh = [pp.tile([128, 64, WP], F16, name=f"xrh{i}") for i in range(2)]
            zt = pp.tile([128, 64, W], F16, name="zt")
            ht = pp.tile([128, 62, W], F16, name="ht")

            def _body():
                # Head of the DMA queue = exactly what the first matmul chunk
                # needs: tap-0 weights, then the first x piece of step 0.
                nc.sync.dma_start(out=w_t[:, 0:1, :],
                                  in_=w_d[0:1].rearrange("t k m -> k t m"))
                nc.sync.dma_start(out=xh[0][0:64, 2:2 + DMA_ROWS, 1:97],
                                  in_=xs_d[:, 0:DMA_ROWS, :])
                nc.sync.dma_start(out=w_t[:, 1:9, :],
                                  in_=w_d[1:9].rearrange("t k m -> k t m"))
                nc.sync.dma_start(out=b_t, in_=b_d)
                nc.sync.dma_start(out=tok_t, in_=tok_d)
                nc.sync.dma_start(out=toko_d, in_=tok_t)

                # Zero fills: pads only. t=0 runs x-only K=64 convs (h == 0),
                # so no h-state zeroing is needed anywhere; the x DMA, the
                # update writes, and the r*h multiply re-cover their interior
                # regions every step before they are read.
                for i in range(2):
                    # x-part top pad rows + W-pad cols (x DMA writes cols 1:97)
                    nc.gpsimd.memset(xh[i][0:64, 0:2, :], 0.0)
                    nc.gpsimd.memset(xh[i][:, :, 0:1], 0.0)
                    nc.gpsimd.memset(xh[i][:, :, 97:98], 0.0)
                    # xrh: x-part top pad row + W-pad cols
                    nc.gpsimd.memset(xrh[i][0:64, 0:1, :], 0.0)
                    nc.gpsimd.memset(xrh[i][:, :, 0:1], 0.0)
                    nc.gpsimd.memset(xrh[i][:, :, 97:98], 0.0)
                    # h-part pad rows (interior is fully rewritten by each
                    # step's update before any read)
                    nc.vector.memset(xh[i][64:128, 0:2, :], 0.0)

                _emit_steps(nc, xs_d, out_d, w_t, b_t, xh, xrh, zt, ht, prz, ph)

            if dynamic_reps:
                tmp = nc.alloc_registers("tmp_reps", mybir.ALL_ENGINES)
                nc.regs_load(tmp, reps_d[0:1, 0:1])
                rv = nc.snap(tmp, donate=True, min_val=1, max_val=1 << 20)
                with tc.For_i(0, rv):
                    _body()
            elif hw_loop and reps > 1:
                with tc.For_i(0, reps):
                    _body()
            else:
                for _rep in range(reps):
                    _body()
    nc.compile()
    return nc


def _emit_steps(nc, xs_d, out_d, w_t, b_t, xh, xrh, zt, ht, prz, ph):
    if True:
        if True:
            for t in range(T):
                R = _r_of(t)
                cur = xh[t % 2]
                nxt = xh[(t + 1) % 2]
                xr = xrh[t % 2]

                # ---- x loads (split into row pieces for DMA parallelism) ----
                off = XOFF[t]
                # XH x-part window rows [2, R+4) <- xs packed rows [off, off+R+2)
                # (t=0 piece 1 was preloaded at the head of the DMA queue)
                r0 = DMA_ROWS if t == 0 else 0
                while r0 < R + 2:
                    r1 = min(r0 + DMA_ROWS, R + 2)
                    nc.sync.dma_start(out=cur[0:64, r0 + 2:r1 + 2, 1:97],
                                      in_=xs_d[:, off + r0:off + r1, :])
                    r0 = r1
                # XRH x-part row j = window row j+1: rows [1, R+2) <- xs [0, R+1)
                r0 = 0
                while r0 < R + 1:
                    r1 = min(r0 + DMA_ROWS, R + 1)
                    nc.gpsimd.dma_start(out=xr[0:64, r0 + 1:r1 + 1, 1:97],
                                        in_=xs_d[:, off + r0:off + r1, :])
                    r0 = r1

                # ---- rz conv: output rows XH [1, 3+R) ----
                # t=0: h == 0, so contract only the x half (K=64); r is not
                # needed (r*h == 0 regardless).
                kk = 64 if t == 0 else 128
                c0 = 1
                while c0 < 3 + R:
                    cr = min(PSUM_ROWS, 3 + R - c0)
                    hh = cr // 2  # rows per bank (cr is always even)
                    pt = prz.tile([128, 2, 512], F32, name="przt", tag="przt")
                    for tap in range(9):
                        di, dj = tap // 3, tap % 3
                        for s in range(2):
                            b0 = c0 + s * hh
                            nc.tensor.matmul(
                                pt[:, s, 0:hh * W],
                                w_t[0:kk, tap, 0:128],
                                cur[0:kk, b0 + di - 1: b0 + di - 1 + hh, dj:dj + W],
                                start=(tap == 0), stop=(tap == 8))
                    if t > 0:
                        # r -> XRH[64:128] rows (c0-1 ..), interior cols
                        nc.scalar.activation(xr[64:128, c0 - 1:c0 - 1 + cr, 1:97],
                                             pt[64:128, :, 0:hh * W], AF.Sigmoid,
                                             bias=b_t[64:128, 0:1])
                    # z -> Zt[64:128] rows (c0-1 ..) [cross-base ACT]
                    nc.scalar.activation(zt[64:128, c0 - 1:c0 - 1 + cr, :],
                                         pt[0:64, :, 0:hh * W], AF.Sigmoid,
                                         bias=b_t[0:64, 0:1])
                    c0 += cr

                # ---- rh = r * h in place in XRH (rows XH [1, 3+R)) ----
                if t > 0:
                    c0 = 1
                    while c0 < 3 + R:
                        cr = min(DVE_ROWS, 3 + R - c0)
                        nc.vector.tensor_tensor(
                            xr[64:128, c0 - 1:c0 - 1 + cr, 1:97],
                            xr[64:128, c0 - 1:c0 - 1 + cr, 1:97],
                            cur[64:128, c0:c0 + cr, 1:97],
                            op=ALU.mult)
                        c0 += cr

                # ---- h-tilde conv: output rows XH [2, 2+R) ----
                c0 = 2
                while c0 < 2 + R:
                    cr = min(PSUM_ROWS, 2 + R - c0)
                    hh = cr // 2
                    pt = ph.tile([128, 2, 512], F32, name="pht", tag="pht")
                    for tap in range(9):
                        di, dj = tap // 3, tap % 3
                        for s in range(2):
                            b0 = c0 + s * hh
                            # XRH row j = XH row j+1: XH row (b0+di-1) -> -1
                            nc.tensor.matmul(
                                pt[0:64, s, 0:hh * W],
                                w_t[0:kk, tap, 128:192],
                                xr[0:kk, b0 + di - 2: b0 + di - 2 + hh, dj:dj + W],
                                start=(tap == 0), stop=(tap == 8))
                    # tanh -> HT[64:128] rows (c0-2 ..) [cross-base ACT]
                    nc.scalar.activation(ht[64:128, c0 - 2:c0 - 2 + cr, :],
                                         pt[0:64, :, 0:hh * W], AF.Tanh,
                                         bias=b_t[0:64, 1:2])
                    c0 += cr

                # ---- elementwise update, rows XH [2, 2+R); store owned rows
                # [2, 50) per chunk so the out DMA overlaps later chunks ----
                # t=0: h == 0 so h_new = z * h~ in one op. t=7: smaller chunks
                # to shorten the final drain dependency chain.
                rows_per = 8 if t == T - 1 else DVE_ROWS
                c0 = 2
                piece = 0
                while c0 < 2 + R:
                    cr = min(rows_per, 2 + R - c0)
                    hrows = slice(c0 - 2, c0 - 2 + cr)   # HT rows
                    zrows = slice(c0 - 1, c0 - 1 + cr)   # Zt rows
                    xrows = slice(c0, c0 + cr)           # XH rows
                    if t == 0:
                        nc.vector.tensor_tensor(
                            nxt[64:128, xrows, 1:97],
                            zt[64:128, zrows, :],
                            ht[64:128, hrows, :], op=ALU.mult)
                    else:
                        # d = h~ - h (in place in HT)
                        nc.vector.tensor_tensor(
                            ht[64:128, hrows, :], ht[64:128, hrows, :],
                            cur[64:128, xrows, 1:97], op=ALU.subtract)
                        # p = z * d (in place in HT)
                        nc.vector.tensor_tensor(
                            ht[64:128, hrows, :], zt[64:128, zrows, :],
                            ht[64:128, hrows, :], op=ALU.mult)
                        # h_new = h + p -> next XH
                        nc.vector.tensor_tensor(
                            nxt[64:128, xrows, 1:97],
                            cur[64:128, xrows, 1:97],
                            ht[64:128, hrows, :], op=ALU.add)
                    c0 += cr

                # ---- store owned rows [2, 50) ----
                for piece in range(2):
                    a = 2 + 24 * piece
                    eng = nc.sync if piece == 0 else nc.gpsimd
                    eng.dma_start(
                        out=out_d[t, :, 24 * piece:24 * piece + 24, :],
                        in_=nxt[64:128, a:a + 24, 1:97])


_NC_CACHE = None


def _get_nc():
    global _NC_CACHE
    if _NC_CACHE is None:
        _NC_CACHE = build_program()
    return _NC_CACHE


def prep_core_inputs(x, w_r, b_r, w_z, b_z, w_h, b_h):
    """Host-side shard prep. Returns list of 8 in_maps."""
    x = np.asarray(x, np.float32).reshape(B, T, C, HW, HW)

    w_rz = np.concatenate([np.asarray(w_z), np.asarray(w_r)], axis=0)  # [128,128,3,3]
    w_hh = np.asarray(w_h)                                             # [64,128,3,3]
    brz = np.concatenate([np.asarray(b_z), np.asarray(b_r)]).astype(np.float32)
    bh = np.asarray(b_h).astype(np.float32)
    bpack = np.zeros((128, 2), np.float32)
    bpack[:, 0] = brz
    bpack[0:64, 1] = bh

    packs = {}
    for flip in (0, 1):
        wrz_f = w_rz[:, :, ::-1, :] if flip else w_rz
        wh_f = w_hh[:, :, ::-1, :] if flip else w_hh
        # [9, K, M]: tap = di*3+dj, entry [k, m] = w[m, k, di, dj]
        wrz_p = wrz_f.transpose(2, 3, 1, 0).reshape(9, 128, 128)
        wh_p = wh_f.transpose(2, 3, 1, 0).reshape(9, 128, 64)
        packs[flip] = np.ascontiguousarray(
            np.concatenate([wrz_p, wh_p], axis=2)).astype(np.float16)

    in_maps = []
    for core in range(NCORES):
        b, flip = core // 2, core % 2
        if flip == 0:
            shard = x[b, :, :, 0:64, :]
        else:
            shard = x[b, :, :, 32:96, :][:, :, ::-1, :]
        # pack per-step shrinking windows: [C, sum_t R_t+2, W]
        packed = np.concatenate(
            [shard[t, :, 0:XOFF[t + 1] - XOFF[t], :] for t in range(T)], axis=1)
        in_maps.append({
            "xs": np.ascontiguousarray(packed, dtype=np.float16),
            "w": packs[flip],
            "b": bpack,
            "tok": np.zeros((1, 1), np.float32),
        })
    return in_maps


def assemble_output(results):
    out = np.empty((B, T, C, HW, HW), np.float32)
    for core in range(NCORES):
        b, flip = core // 2, core % 2
        shard = np.asarray(results[core]["out"], np.float32)  # [T, C, 48, 96]
        if flip == 0:
            out[b, :, :, 0:48, :] = shard
        else:
            out[b, :, :, 48:96, :] = shard[:, :, ::-1, :]
    return out.reshape(B * T, C, HW, HW)


def run_on_hw(inputs, trace=False):
    from concourse.bass_utils import run_bass_kernel_spmd
    nc = _get_nc()
    in_maps = prep_core_inputs(**inputs)
    res = run_bass_kernel_spmd(nc, in_maps, list(range(NCORES)), trace=trace)
    return assemble_output(res.results), res


def kernel(**inputs):
    out, _ = run_on_hw(inputs, trace=False)
    return out



# revision 25
# speedup vs baseline: 1.0761x; 1.0761x over previous
"""ConvGRU Trainium2 kernel (nn_ConvRnn): B=4, T=8, C_in=C_out=64, H=W=96, 3x3 SAME.

Strategy (v2 — fp16 end-to-end to minimize PJRT operand bytes):
- 8 cores = 4 samples x 2 height-halves. Bottom halves are row-flipped on the
  host (weights row-flipped too) so a single SPMD program serves all cores.
- No cross-core communication: each core computes a shrinking extended region
  R_t = 48 + 2*(7-t) rows so the halo needed by later steps is computed
  redundantly (avg +15% compute, zero sync).
- Everything on SBUF is fp16 (x, h, weights, gates); PSUM accumulates fp32 and
  activations apply fp32 biases. Measured axon dispatch cost is ~0.6 ms per MB
  of bound input operands per core, so halving bytes is the dominant win; the
  on-device exec is only a few hundred us.
- x arrives unpadded and row-packed [C,456,96] fp16: step t only reads its
  shrinking window of R_t+2 rows, so per-step slices are concatenated on the
  host (sum_t R_t+2 = 456 < 8*64). Zero padding rows/cols live in SBUF tiles
  zero-filled once by memset (no 'zeros' input tensor).
- Convs are 9 shifted fp16 matmuls (K=128 channels = [x|h], M=out channels,
  N<=512 pixels) accumulating in PSUM. Width padded to 98 with zero columns.
- Layout: channel c of x at partition c (0:64); h/r/rh/z/h_tilde at partition
  64+c so every 2-input DVE op has equal base partitions.
- Per step: rz-conv -> sigmoid (r into XRH, z into Zt) -> rh=r*h in-place in
  XRH -> h-conv -> tanh -> d=h~-h, p=z*d (in-place in HT) -> h_new = h+p into
  next XH; h_new rows [2,50) DMA'd to fp16 output.
"""
import numpy as np

import concourse.bacc as bacc
import concourse.tile as tile
from concourse import mybir

F32 = mybir.dt.float32
F16 = mybir.dt.float16
AF = mybir.ActivationFunctionType
ALU = mybir.AluOpType

B, T, C, HW = 4, 8, 64, 96
W = 96
WP = 98          # padded width
WIN = 66         # rows per shard window (incl. 2 top pad rows)
XOFF = [0]       # per-step row offsets into the packed xs input
for _t in range(8):
    XOFF.append(XOFF[-1] + 50 + 2 * (7 - _t))
XROWS = XOFF[-1]  # 456 packed rows total
NCORES = 8
PSUM_ROWS = 10   # rows per PSUM chunk (2 banks)
DVE_ROWS = 16    # rows per DVE elementwise chunk
DMA_ROWS = 17    # rows per x-load DMA piece


def _r_of(t):
    return 48 + 2 * (7 - t)


def build_program(reps=1, hw_loop=False, dynamic_reps=False):
    """reps>1 repeats the full body (incl. state zero-fill) on-device so a
    timing harness can measure steady-state per-execution HW time as the
    slope of wall time vs reps. hw_loop uses a hardware For_i (constant
    program size; an all-engine barrier separates iterations); dynamic_reps
    makes the loop bound a runtime [1,1] uint32 input named "reps" so one
    NEFF serves every rep count. kernel() always uses reps=1."""
    nc = bacc.Bacc("TRN2", target_bir_lowering=False, debug=False,
                   enable_asserts=False, num_devices=NCORES)
    reps_d = None
    if dynamic_reps:
        hw_loop = True
        reps_d = nc.dram_tensor("reps", [1, 1], mybir.dt.uint32,
                                kind="ExternalInput")
    xs_d = nc.dram_tensor("xs", [C, XROWS, W], F16, kind="ExternalInput").ap()
    w_d = nc.dram_tensor("w", [9, 128, 192], F16, kind="ExternalInput").ap()
    b_d = nc.dram_tensor("b", [128, 2], F32, kind="ExternalInput").ap()
    out_d = nc.dram_tensor("out", [T, C, 48, W], F16, kind="ExternalOutput").ap()
    # Chaining token: lets a timing harness thread a data dependency through
    # repeated executions so XLA cannot CSE/DCE the repeats.
    tok_d = nc.dram_tensor("tok", [1, 1], F32, kind="ExternalInput").ap()
    toko_d = nc.dram_tensor("toko", [1, 1], F32, kind="ExternalOutput").ap()

    with tile.TileContext(nc) as tc:
        with tc.tile_pool(name="persist", bufs=1) as pp, \
             tc.tile_pool(name="prz", bufs=2, space="PSUM") as prz, \
             tc.tile_pool(name="ph", bufs=2, space="PSUM") as ph:
            w_t = pp.tile([128, 9, 192], F16, name="w")
            b_t = pp.tile([128, 2], F32, name="b")
            tok_t = pp.tile([1, 1], F32, name="tok")

            # Persistent double-buffered tiles (explicit, so zero-fill persists)
            xh = [pp.tile([128, WIN, WP], F16, name=f"xh{i}") for i in range(2)]
            xrh = [pp.tile([128, 64, WP], F16, name=f"xrh{i}") for i in range(2)]
            zt = pp.tile([128, 64, W], F16, name="zt")
            ht = pp.tile([128, 62, W], F16, name="ht")

            def _body():
                # Head of the DMA queue = exactly what the first matmul chunk
                # needs: tap-0 weights, then the first x piece of step 0.
                nc.sync.dma_start(out=w_t[:, 0:1, :],
                                  in_=w_d[0:1].rearrange("t k m -> k t m"))
                nc.sync.dma_start(out=xh[0][0:64, 2:2 + DMA_ROWS, 1:97],
                                  in_=xs_d[:, 0:DMA_ROWS, :])
                nc.sync.dma_start(out=w_t[:, 1:9, :],
                                  in_=w_d[1:9].rearrange("t k m -> k t m"))
                nc.sync.dma_start(out=b_t, in_=b_d)
                nc.sync.dma_start(out=tok_t, in_=tok_d)
                nc.sync.dma_start(out=toko_d, in_=tok_t)

                # Zero fills: xh[0] h-part is the t=0 hidden state (split so
                # early rows unblock the first matmul chunk sooner); all other
                # interiors are rewritten every step before they are read, so
                # only pad rows/cols need zeroing.
                nc.vector.memset(xh[0][64:128, 0:16, :], 0.0)
                nc.vector.memset(xh[0][64:128, 16:WIN, :], 0.0)
                for i in range(2):
                    # x-part top pad rows + W-pad cols (x DMA writes cols 1:97)
                    nc.gpsimd.memset(xh[i][0:64, 0:2, :], 0.0)
                    nc.gpsimd.memset(xh[i][:, :, 0:1], 0.0)
                    nc.gpsimd.memset(xh[i][:, :, 97:98], 0.0)
                    # xrh: x-part top pad row + W-pad cols
                    nc.gpsimd.memset(xrh[i][0:64, 0:1, :], 0.0)
                    nc.gpsimd.memset(xrh[i][:, :, 0:1], 0.0)
                    nc.gpsimd.memset(xrh[i][:, :, 97:98], 0.0)
                # xh[1] h-part pad rows (interior is fully rewritten by the
                # t=0 update before any read)
                nc.vector.memset(xh[1][64:128, 0:2, :], 0.0)

                _emit_steps(nc, xs_d, out_d, w_t, b_t, xh, xrh, zt, ht, prz, ph)

            if dynamic_reps:
                tmp = nc.alloc_registers("tmp_reps", mybir.ALL_ENGINES)
                nc.regs_load(tmp, reps_d[0:1, 0:1])
                rv = nc.snap(tmp, donate=True, min_val=1, max_val=1 << 20)
                with tc.For_i(0, rv):
                    _body()
            elif hw_loop and reps > 1:
                with tc.For_i(0, reps):
                    _body()
            else:
                for _rep in range(reps):
                    _body()
    nc.compile()
    return nc


def _emit_steps(nc, xs_d, out_d, w_t, b_t, xh, xrh, zt, ht, prz, ph):
    if True:
        if True:
            for t in range(T):
                R = _r_of(t)
                cur = xh[t % 2]
                nxt = xh[(t + 1) % 2]
                xr = xrh[t % 2]

                # ---- x loads (split into row pieces for DMA parallelism) ----
                off = XOFF[t]
                # XH x-part window rows [2, R+4) <- xs packed rows [off, off+R+2)
                # (t=0 piece 1 was preloaded at the head of the DMA queue)
                r0 = DMA_ROWS if t == 0 else 0
                while r0 < R + 2:
                    r1 = min(r0 + DMA_ROWS, R + 2)
                    nc.sync.dma_start(out=cur[0:64, r0 + 2:r1 + 2, 1:97],
                                      in_=xs_d[:, off + r0:off + r1, :])
                    r0 = r1
                # XRH x-part row j = window row j+1: rows [1, R+2) <- xs [0, R+1)
                r0 = 0
                while r0 < R + 1:
                    r1 = min(r0 + DMA_ROWS, R + 1)
                    nc.gpsimd.dma_start(out=xr[0:64, r0 + 1:r1 + 1, 1:97],
                                        in_=xs_d[:, off + r0:off + r1, :])
                    r0 = r1

                # ---- rz conv: output rows XH [1, 3+R) ----
                kk = 128
                c0 = 1
                while c0 < 3 + R:
                    cr = min(PSUM_ROWS, 3 + R - c0)
                    hh = cr // 2  # rows per bank (cr is always even)
                    pt = prz.tile([128, 2, 512], F32, name="przt", tag="przt")
                    for tap in range(9):
                        di, dj = tap // 3, tap % 3
                        for s in range(2):
                            b0 = c0 + s * hh
                            nc.tensor.matmul(
                                pt[:, s, 0:hh * W],
                                w_t[0:kk, tap, 0:128],
                                cur[0:kk, b0 + di - 1: b0 + di - 1 + hh, dj:dj + W],
                                start=(tap == 0), stop=(tap == 8))
                    # r -> XRH[64:128] rows (c0-1 ..), interior cols
                    nc.scalar.activation(xr[64:128, c0 - 1:c0 - 1 + cr, 1:97],
                                         pt[64:128, :, 0:hh * W], AF.Sigmoid,
                                         bias=b_t[64:128, 0:1])
                    # z -> Zt[64:128] rows (c0-1 ..) [cross-base ACT]
                    nc.scalar.activation(zt[64:128, c0 - 1:c0 - 1 + cr, :],
                                         pt[0:64, :, 0:hh * W], AF.Sigmoid,
                                         bias=b_t[0:64, 0:1])
                    c0 += cr

                # ---- rh = r * h in place in XRH (rows XH [1, 3+R)) ----
                c0 = 1
                while c0 < 3 + R:
                    cr = min(DVE_ROWS, 3 + R - c0)
                    nc.vector.tensor_tensor(
                        xr[64:128, c0 - 1:c0 - 1 + cr, 1:97],
                        xr[64:128, c0 - 1:c0 - 1 + cr, 1:97],
                        cur[64:128, c0:c0 + cr, 1:97],
                        op=ALU.mult)
                    c0 += cr

                # ---- h-tilde conv: output rows XH [2, 2+R) ----
                c0 = 2
                while c0 < 2 + R:
                    cr = min(PSUM_ROWS, 2 + R - c0)
                    hh = cr // 2
                    pt = ph.tile([128, 2, 512], F32, name="pht", tag="pht")
                    for tap in range(9):
                        di, dj = tap // 3, tap % 3
                        for s in range(2):
                            b0 = c0 + s * hh
                            # XRH row j = XH row j+1: XH row (b0+di-1) -> -1
                            nc.tensor.matmul(
                                pt[0:64, s, 0:hh * W],
                                w_t[0:kk, tap, 128:192],
                                xr[0:kk, b0 + di - 2: b0 + di - 2 + hh, dj:dj + W],
                                start=(tap == 0), stop=(tap == 8))
                    # tanh -> HT[64:128] rows (c0-2 ..) [cross-base ACT]
                    nc.scalar.activation(ht[64:128, c0 - 2:c0 - 2 + cr, :],
                                         pt[0:64, :, 0:hh * W], AF.Tanh,
                                         bias=b_t[0:64, 1:2])
                    c0 += cr

                # ---- elementwise update, rows XH [2, 2+R); store owned rows
                # [2, 50) per chunk so the out DMA overlaps later chunks ----
                # t=0: h == 0 so h_new = z * h~ in one op. t=7: smaller chunks
                # to shorten the final drain dependency chain.
                rows_per = 8 if t == T - 1 else DVE_ROWS
                c0 = 2
                piece = 0
                while c0 < 2 + R:
                    cr = min(rows_per, 2 + R - c0)
                    hrows = slice(c0 - 2, c0 - 2 + cr)   # HT rows
                    zrows = slice(c0 - 1, c0 - 1 + cr)   # Zt rows
                    xrows = slice(c0, c0 + cr)           # XH rows
                    # d = h~ - h (in place in HT)
                    nc.vector.tensor_tensor(
                        ht[64:128, hrows, :], ht[64:128, hrows, :],
                        cur[64:128, xrows, 1:97], op=ALU.subtract)
                    # p = z * d (in place in HT)
                    nc.vector.tensor_tensor(
                        ht[64:128, hrows, :], zt[64:128, zrows, :],
                        ht[64:128, hrows, :], op=ALU.mult)
                    # h_new = h + p -> next XH
                    nc.vector.tensor_tensor(
                        nxt[64:128, xrows, 1:97],
                        cur[64:128, xrows, 1:97],
                        ht[64:128, hrows, :], op=ALU.add)
                    c0 += cr

                # ---- store owned rows [2, 50) ----
                for piece in range(2):
                    a = 2 + 24 * piece
                    eng = nc.sync if piece == 0 else nc.gpsimd
                    eng.dma_start(
                        out=out_d[t, :, 24 * piece:24 * piece + 24, :],
                        in_=nxt[64:128, a:a + 24, 1:97])


_NC_CACHE = None


def _get_nc():
    global _NC_CACHE
    if _NC_CACHE is None:
        _NC_CACHE = build_program()
    return _NC_CACHE


def prep_core_inputs(x, w_r, b_r, w_z, b_z, w_h, b_h):
    """Host-side shard prep. Returns list of 8 in_maps."""
    x = np.asarray(x, np.float32).reshape(B, T, C, HW, HW)

    w_rz = np.concatenate([np.asarray(w_z), np.asarray(w_r)], axis=0)  # [128,128,3,3]
    w_hh = np.asarray(w_h)                                             # [64,128,3,3]
    brz = np.concatenate([np.asarray(b_z), np.asarray(b_r)]).astype(np.float32)
    bh = np.asarray(b_h).astype(np.float32)
    bpack = np.zeros((128, 2), np.float32)
    bpack[:, 0] = brz
    bpack[0:64, 1] = bh

    packs = {}
    for flip in (0, 1):
        wrz_f = w_rz[:, :, ::-1, :] if flip else w_rz
        wh_f = w_hh[:, :, ::-1, :] if flip else w_hh
        # [9, K, M]: tap = di*3+dj, entry [k, m] = w[m, k, di, dj]
        wrz_p = wrz_f.transpose(2, 3, 1, 0).reshape(9, 128, 128)
        wh_p = wh_f.transpose(2, 3, 1, 0).reshape(9, 128, 64)
        packs[flip] = np.ascontiguousarray(
            np.concatenate([wrz_p, wh_p], axis=2)).astype(np.float16)

    in_maps = []
    for core in range(NCORES):
        b, flip = core // 2, core % 2
        if flip == 0:
            shard = x[b, :, :, 0:64, :]
        else:
            shard = x[b, :, :, 32:96, :][:, :, ::-1, :]
        # pack per-step shrinking windows: [C, sum_t R_t+2, W]
        packed = np.concatenate(
            [shard[t, :, 0:XOFF[t + 1] - XOFF[t], :] for t in range(T)], axis=1)
        in_maps.append({
            "xs": np.ascontiguousarray(packed, dtype=np.float16),
            "w": packs[flip],
            "b": bpack,
            "tok": np.zeros((1, 1), np.float32),
        })
    return in_maps


def assemble_output(results):
    out = np.empty((B, T, C, HW, HW), np.float32)
    for core in range(NCORES):
        b, flip = core // 2, core % 2
        shard = np.asarray(results[core]["out"], np.float32)  # [T, C, 48, 96]
        if flip == 0:
            out[b, :, :, 0:48, :] = shard
        else:
            out[b, :, :, 48:96, :] = shard[:, :, ::-1, :]
    return out.reshape(B * T, C, HW, HW)


def run_on_hw(inputs, trace=False):
    from concourse.bass_utils import run_bass_kernel_spmd
    nc = _get_nc()
    in_maps = prep_core_inputs(**inputs)
    res = run_bass_kernel_spmd(nc, in_maps, list(range(NCORES)), trace=trace)
    return assemble_output(res.results), res


def kernel(**inputs):
    out, _ = run_on_hw(inputs, trace=False)
    return out

